# revision 19
# baseline (speedup 1.0000x reference)
"""Trainium2 Bass kernel for nn_DecoderWithAttention (adaptive attention LSTM decoder).

Sharding: data-parallel over batch (16 rows/core x 8 cores). Host does
sort/shard/gather/layout glue; device does all matmuls, the sequential LSTM,
attention softmax, and the fc projection.

Device-side layout convention: features on partitions, (time, batch) on the
free axis ("T-layout"). The LSTM recurrence is the only sequential part; the
attention and fc phases are deferred and batched over all 20 timesteps.
"""

import numpy as np
import ml_dtypes
from contextlib import ExitStack

import concourse.bass as bass
import concourse.mybir as mybir
import concourse.tile as tile
from concourse.bass_utils import run_bass_kernel_spmd

F32 = mybir.dt.float32
BF16 = mybir.dt.bfloat16
AF = mybir.ActivationFunctionType

B, P, H, E, V, A, IF, L, T = 128, 49, 512, 512, 10000, 49, 2048, 21, 20
NB = 16          # per-core batch
NCORES = 8
TB = T * NB      # 320 (t,b) columns
VC = 79          # fc V-chunks of 128 (V padded to 10112)
VPAD = VC * 128
HC = H // 128    # 4 h-chunks
GC = 5 * H // 128  # 20 gate chunks
KI = 2 * E // 128  # 8 input K-chunks
IFC = IF // 128  # 16 encoder-feature chunks
HB = HC * NB     # 64
NP_ = NB * P     # 784
NZ = NP_ + NB    # 800 (zt cols b*49+p, sentinel cols 784:800)

_CACHE = {}



def _split_multi_waits(nc):
    """Walrus in this toolchain accepts only one sync-wait per instruction;
    hoist extra waits into same-engine EventSemaphore instructions just
    before the original (engine streams execute in order, so sequential
    waits are equivalent)."""
    n = 0
    for f in nc.m.functions:
        for bb in f.blocks:
            il = bb.instructions
            i = 0
            while i < len(il):
                ins = il[i]
                si = ins.sync_info
                if si is not None and si.on_wait is not None and len(si.on_wait) > 1:
                    extras = list(si.on_wait[:-1])
                    try:
                        si.on_wait = [si.on_wait[-1]]
                    except Exception:
                        ins.sync_info = mybir.SyncInfo(on_wait=[si.on_wait[-1]],
                                                       on_update=si.on_update)
                    for j, w in enumerate(extras):
                        ev = mybir.InstEventSemaphore(
                            name=f"{ins.name}_swait{j}", engine=ins.engine,
                            ins=[], outs=[],
                            sync_info=mybir.SyncInfo(on_wait=[w], on_update=[]))
                        il.insert(i, ev)
                        i += 1
                        n += 1
                i += 1
    return n


def _build_program():
    nc = bass.Bass()

    def din(name, shape, dtype):
        return nc.dram_tensor(name, list(shape), dtype, kind="ExternalInput")

    xt_d = din("xt", (128, KI * TB), BF16)          # X^T tiled: col kc*320 + t*16 + b
    spn_d = din("spn", (49, NB * H), BF16)          # spatial natural: [p, b*512+f]
    spt_d = din("spt", (128, HC * NP_), BF16)       # spatial^T: col kc*784 + b*49+p
    enc_d = din("enc", (IFC, 128, NP_), BF16)       # enc_image^T per kc
    mtb_d = din("mtb", (T, NB), F32)                # active mask [t, b]
    mrow_d = din("mrow", (1, TB), BF16)             # mask row t*16+b
    mrow4_d = din("mrow4", (1, HC * TB), BF16)      # mask row t*64 + fc*16 + b
    wih_d = din("wih", (GC, 128, KI * 128), BF16)   # per mc: [k, kc*128+m]
    whh_d = din("whh", (128, HC * GC * 128), BF16)  # col (kc*20+mc)*128+m
    wfc_d = din("wfc", (VC, 128, HC * 128), BF16)   # per vc: [k, kc*128+m]
    wit_d = din("wit", (HC, 128, IFC * 128), BF16)  # init_h^T/49 per fc: [k, kc*128+m]
    wct_d = din("wct", (HC, 128, IFC * 128), BF16)
    wda_d = din("wda", (128, HC * A), BF16)         # dec_att^T col kc*49+a
    wsa_d = din("wsa", (128, HC * A), BF16)
    wea_d = din("wea", (128, HC * A), BF16)
    v_d = din("v", (A, 1), BF16)
    ones1_d = din("ones1", (1, 128), BF16)
    ident_d = din("ident", (32, 32), F32)
    gbias_d = din("gbias", (128, GC), F32)
    ibias_d = din("ibias", (128, 2 * HC), F32)
    eab_d = din("eab", (A, 1), F32)
    fcb_d = din("fcb", (128, VC), F32)

    predsT_d = nc.dram_tensor("predsT", [VPAD, TB], F32, kind="ExternalOutput")
    alphas_d = nc.dram_tensor("alphas", [T, NB, P], F32, kind="ExternalOutput")
    betas_d = nc.dram_tensor("betas", [T, NB], F32, kind="ExternalOutput")

    with tile.TileContext(nc) as tc, ExitStack() as ctx:
        wp = ctx.enter_context(tc.tile_pool(name="weights", bufs=1))
        sp = ctx.enter_context(tc.tile_pool(name="state", bufs=1))
        tp = ctx.enter_context(tc.tile_pool(name="tmp", bufs=3))
        strm = ctx.enter_context(tc.tile_pool(name="stream", bufs=3))
        fcw = ctx.enter_context(tc.tile_pool(name="fcw", bufs=4))
        att = ctx.enter_context(tc.tile_pool(name="att", bufs=4))
        pmm = ctx.enter_context(tc.tile_pool(name="pmm", bufs=3, space="PSUM"))
        pzp = ctx.enter_context(tc.tile_pool(name="pzp", bufs=2, space="PSUM"))

        def load(pool, dram, shape, dtype, tag=None):
            t = pool.tile(list(shape), dtype, tag=tag or dram.name)
            nc.sync.dma_start(t[:], dram[:])
            return t

        xt = load(wp, xt_d, (128, KI * TB), BF16)
        spn = load(wp, spn_d, (49, NB * H), BF16)
        whh = load(wp, whh_d, (128, HC * GC * 128), BF16)
        wda = load(wp, wda_d, (128, HC * A), BF16)
        wsa = load(wp, wsa_d, (128, HC * A), BF16)
        wea = load(wp, wea_d, (128, HC * A), BF16)
        v_sb = load(wp, v_d, (A, 1), BF16)
        ones1 = load(wp, ones1_d, (1, 128), BF16)
        ident = load(wp, ident_d, (32, 32), F32)
        gbias = load(wp, gbias_d, (128, GC), F32)
        ibias = load(wp, ibias_d, (128, 2 * HC), F32)
        eab = load(wp, eab_d, (A, 1), F32)
        fcb = load(wp, fcb_d, (128, VC), F32)
        mtb = load(wp, mtb_d, (T, NB), F32)
        mrow = load(wp, mrow_d, (1, TB), BF16)
        mrow4 = load(wp, mrow4_d, (1, HC * TB), BF16)

        # ---- mask broadcast tiles (ones1^T @ mrow) ----
        mps = pmm.tile([128, TB], F32, tag="mm")
        nc.tensor.matmul(mps[:], ones1[:], mrow[:], start=True, stop=True)
        mask_bc = sp.tile([128, TB], F32)
        nc.vector.tensor_copy(mask_bc[:], mps[:])
        mask_bc4 = sp.tile([128, HC * TB], F32)
        for o, n in ((0, 512), (512, 512), (1024, 256)):
            m4ps = pmm.tile([128, 512], F32, tag="mm")
            nc.tensor.matmul(m4ps[:, :n], ones1[:], mrow4[:, o:o + n],
                             start=True, stop=True)
            nc.vector.tensor_copy(mask_bc4[:, o:o + n], m4ps[:, :n])

        # ---- Phase B: mean_enc^T via strided free-axis reduce, then h0/c0 ----
        met_f = sp.tile([128, IFC * NB], F32)
        for kc in range(IFC):
            ec = strm.tile([128, NP_], BF16, tag="enc")
            nc.sync.dma_start(ec[:], enc_d[kc])
            nc.vector.reduce_sum(met_f[:, kc * NB:(kc + 1) * NB],
                                 ec[:].rearrange("q (b p) -> q b p", p=P),
                                 axis=mybir.AxisListType.X)
        met = sp.tile([128, IFC * NB], BF16)
        nc.vector.tensor_copy(met[:], met_f[:])

        h_sb = sp.tile([128, HB], F32)
        c_sb = sp.tile([128, HB], F32)
        for half, wt_d in enumerate((wit_d, wct_d)):
            dst = c_sb if half else h_sb
            for fc in range(HC):
                wt = strm.tile([128, IFC * 128], BF16, tag="initw")
                nc.sync.dma_start(wt[:], wt_d[fc])
                hps = pmm.tile([128, NB], F32, tag="mm")
                for kc in range(IFC):
                    nc.tensor.matmul(hps[:], wt[:, kc * 128:(kc + 1) * 128],
                                     met[:, kc * NB:(kc + 1) * NB],
                                     start=(kc == 0), stop=(kc == IFC - 1))
                nc.scalar.activation(dst[:, fc * NB:(fc + 1) * NB], hps[:],
                                     AF.Identity,
                                     bias=ibias[:, half * HC + fc: half * HC + fc + 1])
        h_bf = sp.tile([128, HB], BF16)
        nc.vector.tensor_copy(h_bf[:], h_sb[:])

        # ---- Phase C: G_in = X @ w_ih.T + (b_ih + b_hh), T-layout, bf16 ----
        gin = sp.tile([128, GC * TB], BF16)
        for mc in range(GC):
            wt = strm.tile([128, KI * 128], BF16, tag="wihs")
            nc.sync.dma_start(wt[:], wih_d[mc])
            ps = pmm.tile([128, TB], F32, tag="mm")
            for kc in range(KI):
                nc.tensor.matmul(ps[:], wt[:, kc * 128:(kc + 1) * 128],
                                 xt[:, kc * TB:(kc + 1) * TB],
                                 start=(kc == 0), stop=(kc == KI - 1))
            nc.scalar.activation(gin[:, mc * TB:(mc + 1) * TB], ps[:],
                                 AF.Identity, bias=gbias[:, mc:mc + 1])

        # ---- Phase D: enc_att_out^T = enc_att_W @ spatial^T + b, [49, b*49+p] ----
        ea = sp.tile([A, NP_], F32)
        spt = load(wp, spt_d, (128, HC * NP_), BF16)
        for o, n in ((0, 512), (512, NP_ - 512)):
            ps = pmm.tile([A, 512], F32, tag="mm")
            for kc in range(HC):
                nc.tensor.matmul(ps[:, :n], wea[:, kc * A:(kc + 1) * A],
                                 spt[:, kc * NP_ + o: kc * NP_ + o + n],
                                 start=(kc == 0), stop=(kc == HC - 1))
            nc.scalar.activation(ea[:, o:o + n], ps[:, :n], AF.Identity, bias=eab[:])

        # ---- Phase E: sequential LSTM (T-layout, gates [gate-chunk, b]) ----
        ht_all = sp.tile([128, T * HB], F32)   # h_new (unmasked), col t*64+fc*16+b
        htb_all = sp.tile([128, T * HB], BF16)
        stb_all = sp.tile([128, T * HB], BF16)
        ginv = gin[:].rearrange("q (m t b) -> q m t b", t=T, b=NB)
        for t in range(T):
            ps_g = pmm.tile([128, GC * NB], F32, tag="mm")
            for mc in range(GC):
                for kc in range(HC):
                    nc.tensor.matmul(
                        ps_g[:, mc * NB:(mc + 1) * NB],
                        whh[:, (kc * GC + mc) * 128:(kc * GC + mc + 1) * 128],
                        h_bf[:, kc * NB:(kc + 1) * NB],
                        start=(kc == 0), stop=(kc == HC - 1))
            gts = tp.tile([128, GC * NB], F32, tag="gts")
            nc.vector.tensor_add(
                gts[:].rearrange("q (m b) -> q m b", b=NB),
                ps_g[:].rearrange("q (m b) -> q m b", b=NB),
                ginv[:, :, t])
            acts = tp.tile([128, GC * NB], F32, tag="acts")
            nc.scalar.activation(acts[:, 0:128], gts[:, 0:128], AF.Sigmoid)
            nc.scalar.activation(acts[:, 128:192], gts[:, 128:192], AF.Tanh)
            nc.scalar.activation(acts[:, 192:320], gts[:, 192:320], AF.Sigmoid)
            i_g, f_g, g_g, o_g, s_g = (acts[:, k * HB:(k + 1) * HB] for k in range(5))
            t1 = tp.tile([128, HB], F32, tag="t1")
            t2 = tp.tile([128, HB], F32, tag="t2")
            cn = tp.tile([128, HB], F32, tag="cn")
            tnc = tp.tile([128, HB], F32, tag="tnc")
            nc.vector.tensor_mul(t1[:], f_g, c_sb[:])
            nc.vector.tensor_mul(t2[:], i_g, g_g)
            nc.vector.tensor_add(cn[:], t1[:], t2[:])
            nc.scalar.activation(tnc[:], cn[:], AF.Tanh)
            hn = ht_all[:, t * HB:(t + 1) * HB]
            nc.vector.tensor_mul(hn, o_g, tnc[:])
            nc.vector.tensor_mul(stb_all[:, t * HB:(t + 1) * HB], s_g, tnc[:])
            nc.vector.tensor_copy(htb_all[:, t * HB:(t + 1) * HB], hn)
            mk = mask_bc4[:, t * HB:(t + 1) * HB]
            dh = tp.tile([128, HB], F32, tag="dh")
            nc.vector.tensor_sub(dh[:], hn, h_sb[:])
            nc.vector.tensor_mul(dh[:], dh[:], mk)
            nc.vector.tensor_add(h_sb[:], h_sb[:], dh[:])
            dc = tp.tile([128, HB], F32, tag="dc")
            nc.vector.tensor_sub(dc[:], cn[:], c_sb[:])
            nc.vector.tensor_mul(dc[:], dc[:], mk)
            nc.vector.tensor_add(c_sb[:], c_sb[:], dc[:])
            nc.vector.tensor_copy(h_bf[:], h_sb[:])

        # ---- Phase F: deferred attention ----
        hta = htb_all[:].rearrange("q (t f b) -> q t f b", f=HC, b=NB)
        sta = stb_all[:].rearrange("q (t f b) -> q t f b", f=HC, b=NB)
        hfa = ht_all[:].rearrange("q (t f b) -> q t f b", f=HC, b=NB)
        da = sp.tile([A, TB], F32)
        sa = sp.tile([A, TB], F32)
        for dst, w, src in ((da, wda, hta), (sa, wsa, sta)):
            ps = pmm.tile([A, TB], F32, tag="mm")
            for kc in range(HC):
                nc.tensor.matmul(ps[:], w[:, kc * A:(kc + 1) * A], src[:, :, kc],
                                 start=(kc == 0), stop=(kc == HC - 1))
            nc.vector.tensor_copy(dst[:], ps[:])

        zt_all = sp.tile([T, NZ], F32)
        eav = ea[:].rearrange("a (b p) -> a b p", p=P)
        for t in range(T):
            targ = att.tile([A, NZ], F32, tag="targ")
            nc.vector.tensor_add(
                targ[:, 0:NP_].rearrange("a (b p) -> a b p", p=P),
                eav, da[:, t * NB:(t + 1) * NB].to_broadcast([A, NB, P]))
            nc.vector.tensor_add(targ[:, NP_:NZ], sa[:, t * NB:(t + 1) * NB],
                                 da[:, t * NB:(t + 1) * NB])
            tt = att.tile([A, NZ], BF16, tag="tt")
            nc.scalar.activation(tt[:], targ[:], AF.Tanh)
            zp = pzp.tile([1, NZ], F32, tag="zp")
            for o, n in ((0, 512), (512, NZ - 512)):
                nc.tensor.matmul(zp[:, o:o + n], v_sb[:], tt[:, o:o + n],
                                 start=True, stop=True)
            zrow = att.tile([1, NZ], F32, tag="zrow")
            nc.vector.tensor_copy(zrow[:], zp[:])
            nc.gpsimd.dma_start(zt_all[t:t + 1, :], zrow[:])

        # softmax over p (and p+sentinel) — no max subtraction (|zt| <= ||v||_1)
        e_all = sp.tile([T, NZ], F32)
        nc.scalar.activation(e_all[:], zt_all[:], AF.Exp)
        s49 = sp.tile([T, NB], F32)
        nc.vector.reduce_sum(s49[:], e_all[:, 0:NP_].rearrange("t (b p) -> t b p", p=P),
                             axis=mybir.AxisListType.X)
        den = sp.tile([T, NB], F32)
        nc.vector.tensor_add(den[:], s49[:], e_all[:, NP_:NZ])
        r49 = sp.tile([T, NB], F32)
        r50 = sp.tile([T, NB], F32)
        nc.vector.reciprocal(r49[:], s49[:])
        nc.vector.reciprocal(r50[:], den[:])
        al = sp.tile([T, NP_], F32)
        nc.vector.tensor_mul(al[:].rearrange("t (b p) -> t b p", p=P),
                             e_all[:, 0:NP_].rearrange("t (b p) -> t b p", p=P),
                             r49[:].to_broadcast([T, NB, P]))
        alm = sp.tile([T, NP_], F32)
        nc.vector.tensor_mul(alm[:].rearrange("t (b p) -> t b p", p=P),
                             al[:].rearrange("t (b p) -> t b p", p=P),
                             mtb[:].to_broadcast([T, NB, P]))
        nc.sync.dma_start(alphas_d[:], alm[:].rearrange("t (b p) -> t b p", p=P))
        beta = sp.tile([T, NB], F32)
        nc.vector.tensor_mul(beta[:], e_all[:, NP_:NZ], r50[:])
        betm = tp.tile([T, NB], F32, tag="betm")
        nc.vector.tensor_mul(betm[:], beta[:], mtb[:])
        nc.sync.dma_start(betas_d[:], betm[:])

        # ---- ct^T per batch row: [f, t] = spatial_b^T @ alpha_b^T ----
        ctT = sp.tile([128, HC * TB], F32)  # col fc*320 + t*16 + b
        ctv = ctT[:].rearrange("q (f t b) -> q f t b", f=HC, b=NB)
        for b in range(NB):
            trp = pmm.tile([A, T], F32, tag="mm")
            nc.tensor.transpose(trp[:], al[0:T, b * P:(b + 1) * P], ident[0:T, 0:T])
            atb = tp.tile([A, T], BF16, tag="atb")
            nc.vector.tensor_copy(atb[:], trp[:])
            for fc in range(HC):
                cps = pmm.tile([128, T], F32, tag="mm")
                nc.tensor.matmul(cps[:], spn[:, b * H + fc * 128: b * H + (fc + 1) * 128],
                                 atb[:], start=True, stop=True)
                nc.vector.tensor_copy(ctv[:, fc, :, b], cps[:].rearrange("q t -> q t"))

        # ---- D = beta*s + (1-beta)*ct + h  (bf16, T-layout) ----
        btb = tp.tile([T, NB], BF16, tag="btb")
        nc.vector.tensor_copy(btb[:], beta[:])
        brow = sp.tile([1, TB], BF16)
        nc.gpsimd.dma_start(brow[:], btb[:])
        bps = pmm.tile([128, TB], F32, tag="mm")
        nc.tensor.matmul(bps[:], ones1[:], brow[:], start=True, stop=True)
        bcb = sp.tile([128, TB], F32)
        nc.vector.tensor_copy(bcb[:], bps[:])
        dt_all = sp.tile([128, HC * TB], BF16)
        dtv = dt_all[:].rearrange("q (f t b) -> q f t b", f=HC, b=NB)
        for fc in range(HC):
            t1 = tp.tile([128, TB], F32, tag="d1")
            t1v = t1[:].rearrange("q (t b) -> q t b", b=NB)
            bcv = bcb[:].rearrange("q (t b) -> q t b", b=NB)
            nc.vector.tensor_sub(t1v, sta[:, :, fc], ctv[:, fc])
            nc.vector.tensor_mul(t1v, t1v, bcv)
            nc.vector.tensor_add(t1v, t1v, ctv[:, fc])
            nc.vector.tensor_add(dtv[:, fc], t1v, hfa[:, :, fc])

        # ---- fc: preds^T chunks = fc_W_chunk @ D + fc_b, masked ----
        for vc in range(VC):
            wt = fcw.tile([128, HC * 128], BF16, tag="fcw")
            nc.sync.dma_start(wt[:], wfc_d[vc])
            ps = pmm.tile([128, TB], F32, tag="mm")
            for kc in range(HC):
                nc.tensor.matmul(ps[:], wt[:, kc * 128:(kc + 1) * 128],
                                 dt_all[:, kc * TB:(kc + 1) * TB],
                                 start=(kc == 0), stop=(kc == HC - 1))
            pf = tp.tile([128, TB], F32, tag="pf")
            nc.scalar.activation(pf[:], ps[:], AF.Identity, bias=fcb[:, vc:vc + 1])
            po = tp.tile([128, TB], F32, tag="po")
            nc.vector.tensor_mul(po[:], pf[:], mask_bc[:])
            nc.sync.dma_start(predsT_d[vc * 128:(vc + 1) * 128, :], po[:])

    return nc


def _tile_lhsT(wT, kchunks, mtotal):
    """[K, M] -> [128, kchunks * M] with col kc*M + m, partition = K within chunk."""
    Kdim, M = wT.shape
    assert Kdim == kchunks * 128 and M == mtotal
    return np.ascontiguousarray(
        wT.reshape(kchunks, 128, M).transpose(1, 0, 2).reshape(128, kchunks * M))


def _prep_weights(inp):
    bf = ml_dtypes.bfloat16
    w = {}
    w_ihT = np.asarray(inp["w_ih"], np.float32).T        # [1024, 2560]
    w_hhT = np.asarray(inp["w_hh"], np.float32).T        # [512, 2560]
    fc_WT = np.asarray(inp["fc_W"], np.float32).T        # [512, 10000]
    fc_WTp = np.zeros((H, VPAD), np.float32)
    fc_WTp[:, :V] = fc_WT
    # wih[mc] = [128, kc*128+m]
    w["wih"] = np.ascontiguousarray(
        w_ihT.reshape(KI, 128, GC, 128).transpose(2, 1, 0, 3)
        .reshape(GC, 128, KI * 128).astype(bf))
    w["whh"] = _tile_lhsT(w_hhT.astype(bf), HC, 5 * H)
    w["wfc"] = np.ascontiguousarray(
        fc_WTp.reshape(HC, 128, VC, 128).transpose(2, 1, 0, 3)
        .reshape(VC, 128, HC * 128).astype(bf))
    for nm, key in (("wit", "init_h_W"), ("wct", "init_c_W")):
        wT = (np.asarray(inp[key], np.float32) / P).T    # [2048, 512], mean scale folded
        w[nm] = np.ascontiguousarray(
            wT.reshape(IFC, 128, HC, 128).transpose(2, 1, 0, 3)
            .reshape(HC, 128, IFC * 128).astype(bf))
    for nm, key in (("wda", "dec_att_W"), ("wsa", "sent_att_W"), ("wea", "enc_att_W")):
        wT = np.asarray(inp[key], np.float32).T          # [512, 49]
        w[nm] = _tile_lhsT(wT.astype(bf), HC, A)
    w["v"] = np.asarray(inp["att_out_W"], np.float32)[0].reshape(A, 1).astype(bf)
    w["ones1"] = np.ones((1, 128), bf)
    w["ident"] = np.eye(32, dtype=np.float32)
    w["gbias"] = np.ascontiguousarray(
        (np.asarray(inp["b_ih"], np.float32) + np.asarray(inp["b_hh"], np.float32))
        .reshape(GC, 128).T)
    ib = np.empty((128, 2 * HC), np.float32)
    ib[:, :HC] = np.asarray(inp["init_h_b"], np.float32).reshape(HC, 128).T
    ib[:, HC:] = np.asarray(inp["init_c_b"], np.float32).reshape(HC, 128).T
    w["ibias"] = ib
    w["eab"] = np.asarray(inp["enc_att_b"], np.float32).reshape(A, 1)
    fcbp = np.zeros(VPAD, np.float32)
    fcbp[:V] = np.asarray(inp["fc_b"], np.float32)
    w["fcb"] = np.ascontiguousarray(fcbp.reshape(VC, 128).T)
    return w


def kernel(spatial_f, global_f, enc_image, encoded_captions, caption_lengths,
           emb_W, w_ih, w_hh, b_ih, b_hh, fc_W, fc_b,
           enc_att_W, enc_att_b, dec_att_W, sent_att_W, att_out_W,
           init_h_W, init_h_b, init_c_W, init_c_b):
    bf = ml_dtypes.bfloat16
    inp = dict(w_ih=w_ih, w_hh=w_hh, b_ih=b_ih, b_hh=b_hh, fc_W=fc_W, fc_b=fc_b,
               enc_att_W=enc_att_W, enc_att_b=enc_att_b, dec_att_W=dec_att_W,
               sent_att_W=sent_att_W, att_out_W=att_out_W, init_h_W=init_h_W,
               init_h_b=init_h_b, init_c_W=init_c_W, init_c_b=init_c_b)

    cl = np.asarray(caption_lengths).reshape(-1).astype(np.int64)
    sort_ind = np.argsort(-cl, kind="stable")
    cl_s = cl[sort_ind]
    dl = cl_s - 1
    sp_s = np.asarray(spatial_f, np.float32)[sort_ind]
    gf_s = np.asarray(global_f, np.float32)[sort_ind]
    en_s = np.asarray(enc_image, np.float32)[sort_ind]
    cap_s = np.asarray(encoded_captions)[sort_ind]
    embW = np.asarray(emb_W, np.float32)

    w = _prep_weights(inp)
    in_maps = []
    for c in range(NCORES):
        sl = slice(c * NB, (c + 1) * NB)
        spc, gfc, enc, capc, dlc = sp_s[sl], gf_s[sl], en_s[sl], cap_s[sl], dl[sl]
        m = dict(w)
        xt = np.empty((2 * E, TB), np.float32)   # [1024, t*16+b]
        emb = embW[capc[:, :T].astype(np.int64)]  # [16, 20, 512]
        xt[:E] = emb.transpose(2, 1, 0).reshape(E, TB)
        xt[E:] = np.repeat(gfc.T[:, None, :], T, axis=1).reshape(E, TB)
        m["xt"] = _tile_lhsT(xt.astype(bf), KI, TB)
        m["spn"] = np.ascontiguousarray(
            spc.transpose(1, 0, 2).reshape(P, NB * H).astype(bf))
        m["spt"] = _tile_lhsT(spc.reshape(NB * P, H).T.astype(bf), HC, NB * P)
        m["enc"] = np.ascontiguousarray(
            enc.reshape(NB * P, IF).T.reshape(IFC, 128, NB * P).astype(bf))
        mask = (np.arange(T)[:, None] < dlc[None, :]).astype(np.float32)
        m["mtb"] = mask
        m["mrow"] = mask.reshape(1, TB).astype(bf)
        m["mrow4"] = np.ascontiguousarray(
            np.repeat(mask[:, None, :], HC, axis=1).reshape(1, HC * TB)).astype(bf)
        in_maps.append(m)

    if "nc" not in _CACHE:
        nc = _build_program()
        _split_multi_waits(nc)   # walrus here allows 1 sync-wait per instruction
        _CACHE["nc"] = nc
    import time as _time
    _t0 = _time.perf_counter()
    res = run_bass_kernel_spmd(_CACHE["nc"], in_maps, list(range(NCORES)))
    _CACHE["exec_wall_s"] = _time.perf_counter() - _t0
    _CACHE["last"] = res

    preds = np.zeros((B, T, V), np.float32)
    alphas = np.zeros((B, T, P), np.float32)
    betas = np.zeros((B, T, 1), np.float32)
    for c in range(NCORES):
        r = res.results[c]
        sl = slice(c * NB, (c + 1) * NB)
        preds[sl] = r["predsT"].reshape(VPAD, T, NB).transpose(2, 1, 0)[:, :, :V]
        alphas[sl] = r["alphas"].transpose(1, 0, 2)
        betas[sl] = r["betas"].T[:, :, None]
    return (preds, alphas, betas, cap_s.astype(np.int32), dl.astype(np.int32),
            sort_ind.astype(np.int32))
